# revision 13
# baseline (speedup 1.0000x reference)
"""Distributed Bass attention kernel for 8 TRN2 NeuronCores.

Sharding (zero output-collective): core c handles batch b=c//2, heads
(c%2)*8..+8 over ALL tokens; causal attention computed in scores^T layout
([key, q]) with denominators via an appended ones-row in V; a pairwise
AllToAll exchanges normalized z so each core applies W_O for its half of
the tokens and writes a disjoint output slice.

All matmuls run in bf16 (fp32 PSUM accumulation); softmax exp in fp32 on
the scalar engine. Relative error vs the fp32 reference lands ~1e-3.
"""

import numpy as np
import ml_dtypes

import concourse.bass as bass  # noqa: F401  (AP types pulled transitively)
import concourse.mybir as mybir
import concourse.tile as tile
from concourse import bacc
from concourse.bass_utils import run_bass_kernel_spmd

BF16 = mybir.dt.bfloat16
F32 = mybir.dt.float32
AF = mybir.ActivationFunctionType

B, S, D, H, DH = 4, 2048, 1024, 16, 64
NCORES = 8
HPC = 8           # heads per core
NPAIR = HPC // 2  # head pairs per core
QS = 512          # q supertile
NQS = S // QS
KCH = 128         # key chunk
NKC = S // KCH
TOKH = S // 2     # tokens per core output (half a batch)
FLOC = HPC * DH   # 512 local f-columns


def build():
    nc = bacc.Bacc(None, target_bir_lowering=False, debug=False, num_devices=NCORES)

    xT_e = nc.dram_tensor("xT", [D, S], BF16, kind="ExternalInput")
    wq_e = nc.dram_tensor("wq", [D, FLOC], BF16, kind="ExternalInput")
    wk_e = nc.dram_tensor("wk", [D, FLOC], BF16, kind="ExternalInput")
    wv_e = nc.dram_tensor("wv", [D, FLOC], BF16, kind="ExternalInput")
    wo_e = nc.dram_tensor("wo", [D, D], BF16, kind="ExternalInput")
    out_e = nc.dram_tensor("out", [TOKH, D], F32, kind="ExternalOutput")

    sel_e = nc.dram_tensor("sel", [128, 2], F32, kind="ExternalInput")
    ag_in = nc.dram_tensor("ag_in", [FLOC, S], BF16)
    ag_out = nc.dram_tensor("ag_out", [2, FLOC, S], BF16)

    with tile.TileContext(nc) as tc:
        with (
            tc.tile_pool(name="persist", bufs=1) as PP,
            tc.tile_pool(name="xc", bufs=2) as XP,
            tc.tile_pool(name="exp", bufs=3) as EP,
            tc.tile_pool(name="rows", bufs=2) as RP,
            tc.tile_pool(name="zt", bufs=3) as ZP,
        ):
            # ---- persistent tiles ----
            wq_sb = PP.tile([128, 8 * FLOC], BF16, name="wq_sb")
            wk_sb = PP.tile([128, 8 * FLOC], BF16, name="wk_sb")
            wv_sb = PP.tile([128, 8 * FLOC], BF16, name="wv_sb")
            for c in range(8):
                nc.sync.dma_start(out=wq_sb[:, c * FLOC:(c + 1) * FLOC],
                                  in_=wq_e[c * 128:(c + 1) * 128, :])
                nc.sync.dma_start(out=wk_sb[:, c * FLOC:(c + 1) * FLOC],
                                  in_=wk_e[c * 128:(c + 1) * 128, :])
                nc.sync.dma_start(out=wv_sb[:, c * FLOC:(c + 1) * FLOC],
                                  in_=wv_e[c * 128:(c + 1) * 128, :])

            qt = [PP.tile([128, S], BF16, name=f"qt{p}") for p in range(NPAIR)]
            kt = [PP.tile([128, S], BF16, name=f"kt{p}") for p in range(NPAIR)]
            va = [PP.tile([128, HPC * 65], BF16, name=f"va{k}") for k in range(NKC)]
            for k in range(NKC):
                ones_view = va[k].rearrange("p (u e) -> p u e", u=HPC)[:, :, 64:65]
                nc.vector.memset(ones_view, 1.0)

            ones1 = PP.tile([1, 64], BF16, name="ones1")
            nc.vector.memset(ones1, 1.0)

            maskt = [PP.tile([128, QS], BF16, name=f"maskt{d}") for d in range(4)]
            for d in range(4):
                nc.gpsimd.memset(maskt[d], 1.0)
                nc.gpsimd.affine_select(
                    out=maskt[d], in_=maskt[d],
                    compare_op=mybir.AluOpType.is_ge,
                    fill=0.0, base=-128 * d,
                    pattern=[[1, QS]], channel_multiplier=-1,
                )

            # ---- projections ----
            proj_ctx = tc.tile_pool(name="psproj", bufs=2, space="PSUM")
            PSJ = proj_ctx.__enter__()
            for ts in range(NQS):
                xc = []
                for c in range(8):
                    t = XP.tile([128, QS], BF16, name=f"xc{c}")
                    nc.sync.dma_start(out=t, in_=xT_e[c * 128:(c + 1) * 128,
                                                      ts * QS:(ts + 1) * QS])
                    xc.append(t)
                for p in range(NPAIR):
                    pq = PSJ.tile([128, QS], F32, tag="pq")
                    pk = PSJ.tile([128, QS], F32, tag="pk")
                    for c in range(8):
                        w_off = c * FLOC + p * 128
                        nc.tensor.matmul(pq, lhsT=wq_sb[:, w_off:w_off + 128],
                                         rhs=xc[c], start=(c == 0), stop=(c == 7))
                        nc.tensor.matmul(pk, lhsT=wk_sb[:, w_off:w_off + 128],
                                         rhs=xc[c], start=(c == 0), stop=(c == 7))
                    nc.scalar.copy(qt[p][:, ts * QS:(ts + 1) * QS], pq)
                    nc.scalar.copy(kt[p][:, ts * QS:(ts + 1) * QS], pk)
                for tt in range(4):
                    kci = ts * 4 + tt
                    pv = PSJ.tile([128, QS], F32, tag="pv")
                    for c in range(8):
                        nc.tensor.matmul(pv, lhsT=xc[c][:, tt * 128:(tt + 1) * 128],
                                         rhs=wv_sb[:, c * FLOC:(c + 1) * FLOC],
                                         start=(c == 0), stop=(c == 7))
                    v_view = va[kci].rearrange("p (u e) -> p u e", u=HPC)[:, :, 0:64]
                    nc.scalar.copy(v_view, pv.rearrange("p (u e) -> p u e", u=HPC))

            proj_ctx.__exit__(None, None, None)

            # ---- attention ----
            attn_ctx1 = tc.tile_pool(name="pssc", bufs=2, space="PSUM")
            attn_ctx2 = tc.tile_pool(name="psz", bufs=1, space="PSUM")
            attn_ctx3 = tc.tile_pool(name="psb", bufs=1, space="PSUM")
            PSS = attn_ctx1.__enter__()
            PSZ = attn_ctx2.__enter__()
            PSB = attn_ctx3.__enter__()
            for p in range(NPAIR):
                for qs in range(NQS):
                    nvis = 4 * (qs + 1)
                    zps = [PSZ.tile([65, QS], F32, tag=f"z{u}", name=f"z{u}")
                           for u in range(2)]
                    for kc in range(nvis):
                        sA = PSS.tile([128, QS], F32, tag="sA")
                        sB = PSS.tile([128, QS], F32, tag="sB")
                        nc.tensor.matmul(
                            sA, lhsT=kt[p][0:64, kc * 128:(kc + 1) * 128],
                            rhs=qt[p][0:64, qs * QS:(qs + 1) * QS],
                            start=True, stop=True, tile_position=(0, 0))
                        nc.tensor.matmul(
                            sB, lhsT=kt[p][64:128, kc * 128:(kc + 1) * 128],
                            rhs=qt[p][64:128, qs * QS:(qs + 1) * QS],
                            start=True, stop=True, tile_position=(64, 0))
                        eA = EP.tile([128, QS], BF16, tag="eA")
                        eB = EP.tile([128, QS], BF16, tag="eB")
                        nc.scalar.activation(eA, sA, AF.Exp, scale=0.125)
                        nc.scalar.activation(eB, sB, AF.Exp, scale=0.125)
                        dlt = kc - 4 * qs
                        if 0 <= dlt <= 3:
                            nc.vector.tensor_mul(eA, eA, maskt[dlt])
                            nc.vector.tensor_mul(eB, eB, maskt[dlt])
                        for u in range(2):
                            uu = p * 2 + u
                            nc.tensor.matmul(
                                zps[u], lhsT=va[kc][:, uu * 65:uu * 65 + 65],
                                rhs=(eA if u == 0 else eB),
                                start=(kc == 0), stop=(kc == nvis - 1))
                    for u in range(2):
                        den = RP.tile([1, QS], F32, tag=f"den{u}")
                        nc.scalar.copy(den, zps[u][64:65, :])
                        rec = RP.tile([1, QS], F32, tag=f"rec{u}")
                        nc.vector.reciprocal_approx_fast(out=rec, in_=den)
                        recb = RP.tile([1, QS], BF16, tag=f"recb{u}")
                        nc.scalar.copy(recb, rec)
                        bc = PSB.tile([64, QS], F32, tag=f"bc{u}")
                        nc.tensor.matmul(bc, lhsT=ones1, rhs=recb,
                                         start=True, stop=True)
                        bcs = ZP.tile([64, QS], F32, tag=f"bcs{u}")
                        nc.scalar.copy(bcs, bc)
                        zt_t = ZP.tile([64, QS], BF16, tag=f"zt{u}")
                        nc.vector.tensor_mul(zt_t, zps[u][0:64, :], bcs)
                        frow = p * 128 + u * 64
                        nc.sync.dma_start(
                            out=ag_in[frow:frow + 64, qs * QS:(qs + 1) * QS],
                            in_=zt_t)

            attn_ctx3.__exit__(None, None, None)
            attn_ctx2.__exit__(None, None, None)
            attn_ctx1.__exit__(None, None, None)

            # ---- exchange z within batch pairs ----
            nc.gpsimd.collective_compute(
                "AllGather", mybir.AluOpType.bypass,
                replica_groups=[[0, 1], [2, 3], [4, 5], [6, 7]],
                ins=[ag_in.ap().opt()],
                outs=[ag_out.ap().opt()],
            )

            # ---- W_O (token-half selected via per-core 0/1 sel vector) ----
            sel_sb = PP.tile([128, 2], F32, name="sel_sb")
            nc.sync.dma_start(out=sel_sb, in_=sel_e[:, :])
            wo_sb = [PP.tile([128, D], BF16, name=f"wo{fc}") for fc in range(8)]
            ztf = [PP.tile([128, TOKH], BF16, name=f"ztf{fc}") for fc in range(8)]
            for fc in range(8):
                nc.sync.dma_start(out=wo_sb[fc],
                                  in_=wo_e[fc * 128:(fc + 1) * 128, :])
                zf = ZP.tile([128, S], BF16, tag="zfull", name="zfull")
                nc.sync.dma_start(
                    out=zf,
                    in_=ag_out[fc // 4, (fc % 4) * 128:(fc % 4) * 128 + 128, :])
                t1 = ZP.tile([128, TOKH], BF16, tag="selt1", name="selt1")
                nc.vector.tensor_scalar_mul(t1, zf[:, 0:TOKH], sel_sb[:, 0:1])
                t2 = ZP.tile([128, TOKH], BF16, tag="selt2", name="selt2")
                nc.vector.tensor_scalar_mul(t2, zf[:, TOKH:S], sel_sb[:, 1:2])
                nc.vector.tensor_tensor(ztf[fc], t1, t2, op=mybir.AluOpType.add)
            wo_ctx = tc.tile_pool(name="pswo", bufs=2, space="PSUM")
            PSW = wo_ctx.__enter__()
            for tt in range(TOKH // 128):
                po = PSW.tile([128, D], F32, tag="po")
                for fc in range(8):
                    lt = ztf[fc][:, tt * 128:(tt + 1) * 128]
                    nc.tensor.matmul(po[:, 0:512], lhsT=lt, rhs=wo_sb[fc][:, 0:512],
                                     start=(fc == 0), stop=(fc == 7))
                    nc.tensor.matmul(po[:, 512:1024], lhsT=lt, rhs=wo_sb[fc][:, 512:1024],
                                     start=(fc == 0), stop=(fc == 7))
                po_sb = ZP.tile([128, D], F32, tag="posb", name="posb")
                nc.scalar.copy(po_sb, po)
                nc.sync.dma_start(out=out_e[tt * 128:(tt + 1) * 128, :], in_=po_sb)
            wo_ctx.__exit__(None, None, None)

    nc.finalize()
    return nc


_NC = None


def _get_nc():
    global _NC
    if _NC is None:
        _NC = build()
    return _NC


def kernel(x, W_K, W_Q, W_V, W_O):
    bf = ml_dtypes.bfloat16
    x = np.asarray(x, np.float32)
    W_K = np.asarray(W_K, np.float32)
    W_Q = np.asarray(W_Q, np.float32)
    W_V = np.asarray(W_V, np.float32)
    W_O = np.asarray(W_O, np.float32)

    xT = np.ascontiguousarray(np.transpose(x, (0, 2, 1))).astype(bf)  # [B, D, S]

    def wslice(W, c):
        hs = slice((c % 2) * HPC, (c % 2) * HPC + HPC)
        return np.ascontiguousarray(
            np.transpose(W[hs], (2, 0, 1)).reshape(D, FLOC)).astype(bf)

    WOT = np.ascontiguousarray(W_O.T).astype(bf)

    in_maps = []
    for c in range(NCORES):
        b, half = c // 2, c % 2
        sel = np.zeros((128, 2), np.float32)
        sel[:, half] = 1.0
        in_maps.append({
            "xT": np.ascontiguousarray(xT[b]),
            "wq": wslice(W_Q, c),
            "wk": wslice(W_K, c),
            "wv": wslice(W_V, c),
            "wo": WOT,
            "sel": sel,
        })

    res = run_bass_kernel_spmd(_get_nc(), in_maps, core_ids=list(range(NCORES)))
    kernel.last = res

    out = np.empty((B, S, D), np.float32)
    for c in range(NCORES):
        b, half = c // 2, c % 2
        out[b, half * TOKH:(half + 1) * TOKH, :] = res.results[c]["out"]
    return out
